# revision 6
# baseline (speedup 1.0000x reference)
"""Trainium2 Bass kernel for nn_FCVQVae (FC VQ-VAE with rotation trick).

Strategy: data-parallel over batch B=8 across 8 NeuronCores (one batch
element per core). Weights (~90MB fp32) are replicated and streamed from
HBM layer by layer, overlapped with compute via Tile double buffering.
Activations are kept channels-first (channels on partitions, time on the
free axis) so every linear layer is `out = W.T @ X` with W in its natural
(din, dout) layout as the stationary operand.

The VQ nearest-neighbour lookup runs on-device (argmax of 2*x.c - |c|^2 via
DVE max/max_index, codebook gather via indirect DMA); the rotation trick is
done with per-partition vector ops plus two 64x64 PE transposes. Each core
returns its reconstruction (63, 512), VQ indices (64,), and per-row squared
commit error; the host reduces commit loss / perplexity (O(512) scalar work)
and reassembles the full (8, 512, 63) output.
"""

import sys

import numpy as np

sys.path.insert(0, "/opt/trn_rl_repo")

import concourse.bass as bass  # noqa: E402
import concourse.bacc as bacc  # noqa: E402
import concourse.mybir as mybir  # noqa: E402
import concourse.tile as tile  # noqa: E402
from concourse.bass_utils import run_bass_kernel_spmd  # noqa: E402
from concourse.masks import make_identity  # noqa: E402

F32 = mybir.dt.float32
F32R = mybir.dt.float32r
U32 = mybir.dt.uint32
AF = mybir.ActivationFunctionType
ALU = mybir.AluOpType

NFEATS, T0, H, D, CODE_NUM, DOWN, DEPTH, B = 63, 512, 512, 64, 512, 3, 3, 8
NCORES = 8
P = 128

# Matmul dtype mode: "f32" (exact, 4 cyc/row) or "f32r" (1 cyc/row at N>=256).
import os as _os
MM_MODE = _os.environ.get("KERNEL_MM_MODE", "f32")


def _ceil_div(a, b):
    return (a + b - 1) // b


def _layer_specs():
    """Execution-ordered linear layers: (din, dout, T_out, kind).

    kind: relu | linear | resadd | outproj | decin | uplin | outproj2
    """
    specs = []
    specs.append((NFEATS, H, T0, "relu"))                 # enc in_proj
    t = T0
    for _ in range(DOWN):
        t //= 2
        specs.append((2 * H, H, t, "downlin"))            # enc down lin
        for _ in range(DEPTH):
            specs.append((H, 2 * H, t, "fc1"))            # relu-in, relu evac
            specs.append((2 * H, H, t, "resadd"))         # fc2 + residual
    specs.append((H, D, D, "outproj"))                    # enc out_proj (T=L=64)
    specs.append((D, H, D, "decin"))                      # dec in_proj
    l = D
    for _ in range(DOWN):
        for _ in range(DEPTH):
            specs.append((H, 2 * H, l, "fc1"))
            specs.append((2 * H, H, l, "resadd"))
        specs.append((H, 2 * H, l, "uplin"))              # dec up lin + upsample
        l *= 2
    specs.append((H, H, T0, "relu"))                      # dec out_proj1
    specs.append((H, NFEATS, T0, "outproj2"))             # dec out_proj2
    return specs


_SPECS = _layer_specs()

# bias-pack column offsets per layer (one column per 128-wide m-tile)
_BCOLS = []
_nb = 0
for _din, _dout, _t, _kind in _SPECS:
    _BCOLS.append(_nb)
    _nb += _ceil_div(_dout, P)
_NBCOLS = _nb


def _build_program(repeats=1, hw_loop=1):
    nc = bacc.Bacc(None, target_bir_lowering=False, debug=False)
    AT = F32R if MM_MODE == "f32r" else F32

    x0_d = nc.dram_tensor("x0", [NFEATS, T0], AT, kind="ExternalInput")
    cb_d = nc.dram_tensor("cb", [CODE_NUM, D], F32, kind="ExternalInput")
    cbaug_d = nc.dram_tensor("cbaug", [D + 1, CODE_NUM], F32, kind="ExternalInput")
    bpack_d = nc.dram_tensor("bpack", [P, _NBCOLS], F32, kind="ExternalInput")
    w_drams = [
        nc.dram_tensor(f"w{i}", [s[0], s[1]], AT, kind="ExternalInput")
        for i, s in enumerate(_SPECS)
    ]
    y_d = nc.dram_tensor("y", [NFEATS, T0], F32, kind="ExternalOutput")
    idx_d = nc.dram_tensor("idx", [D, 1], U32, kind="ExternalOutput")
    sq_d = nc.dram_tensor("sq", [D, 1], F32, kind="ExternalOutput")


    with tile.TileContext(nc) as tc:
        with (
            tc.tile_pool(name="const", bufs=1) as const,
            tc.tile_pool(name="wp", bufs=3) as wp,
            tc.tile_pool(name="act", bufs=2) as act,
            tc.tile_pool(name="vq", bufs=1) as vq,
            tc.tile_pool(name="ps", bufs=8, space="PSUM") as psp,
        ):
            bpack_sb = const.tile([P, _NBCOLS], F32)
            nc.sync.dma_start(out=bpack_sb[:], in_=bpack_d[:])
            cbaug_sb = const.tile([D + 1, CODE_NUM], F32)
            nc.sync.dma_start(out=cbaug_sb[:], in_=cbaug_d[:])
            ident = const.tile([P, P], F32)
            make_identity(nc, ident[:])

            def emit_all():
                for _rep in range(repeats):
                    _emit_forward(
                        nc, tc, wp, act, vq, psp,
                        bpack_sb, cbaug_sb, ident,
                        x0_d, cb_d, w_drams, y_d, idx_d, sq_d, AT,
                    )

            if hw_loop > 1:
                with tc.For_i(0, hw_loop, 1):
                    emit_all()
            else:
                emit_all()

    nc.compile()
    return nc


def _emit_forward(nc, tc, wp, act, vq, psp, bpack_sb, cbaug_sb, ident,
                  x0_d, cb_d, w_drams, y_d, idx_d, sq_d, AT):
    x0_sb = act.tile([NFEATS, T0], AT, tag="x0")
    nc.sync.dma_start(out=x0_sb[:], in_=x0_d[:])

    def load_w(i):
        din, dout, _, _ = _SPECS[i]
        nk = _ceil_div(din, P)
        if din <= P:
            w_sb = wp.tile([din, dout], AT, tag="w", name=f"w_sb{i}")
            nc.sync.dma_start(out=w_sb[:], in_=w_drams[i][:])
            w_sb = w_sb[:].rearrange("p (k m) -> p k m", k=1)
        else:
            assert din % P == 0
            w_sb = wp.tile([P, nk, dout], AT, tag="w", name=f"w_sb{i}")
            nc.sync.dma_start(
                out=w_sb[:], in_=w_drams[i][:].rearrange("(k p) m -> p k m", p=P)
            )
        return w_sb, nk

    def bias_ap(i, m, mp):
        return bpack_sb[:mp, _BCOLS[i] + m : _BCOLS[i] + m + 1]

    def mm_group(i, rhs_fn, T):
        """Yields (m, mp, psum_tile) for layer i with rhs tiles from rhs_fn(k, kp)."""
        din, dout, _, _ = _SPECS[i]
        w_sb, nk = load_w(i)
        for m in range(_ceil_div(dout, P)):
            mp = min(P, dout - m * P)
            ps = psp.tile([P, T], F32, tag="ps", name=f"ps{i}_{m}")
            for k in range(nk):
                kp = min(P, din - k * P)
                lhsT = w_sb[:kp, k, m * P : m * P + mp]
                rhs = rhs_fn(k, kp)
                nc.tensor.matmul(
                    ps[:mp, :], lhsT, rhs,
                    start=(k == 0), stop=(k == nk - 1),
                )
            yield m, mp, ps

    li = 0  # layer index

    # ---------------- encoder ----------------
    # in_proj (63 -> 512), relu
    T = T0
    X = act.tile([P, H // P, T], AT, tag="X", name="X_inp")
    for m, mp, ps in mm_group(li, lambda k, kp: x0_sb[:], T):
        nc.scalar.activation(X[:mp, m, :], ps[:mp, :], AF.Relu, bias=bias_ap(li, m, mp))
    li += 1

    for _lay in range(DOWN):
        Tin, T = T, T // 2
        # down lin (1024 -> 512): k<4 reads even time cols, k>=4 odd
        def rhs_down(k, kp, _X=X, _Tin=Tin):
            half = k // (H // P)
            blk = k % (H // P)
            return _X[:, blk, :].rearrange("p (t two) -> p two t", two=2)[:, half, :]

        Xn = act.tile([P, H // P, T], AT, tag="X", name=f"X_dn{li}")
        for m, mp, ps in mm_group(li, rhs_down, T):
            nc.scalar.activation(
                Xn[:mp, m, :], ps[:mp, :], AF.Identity, bias=bias_ap(li, m, mp)
            )
        X = Xn
        li += 1

        for _blk in range(DEPTH):
            XR = act.tile([P, H // P, T], AT, tag="XR", name=f"XR{li}")
            nc.vector.tensor_scalar_max(XR[:], X[:], 0.0)
            Hh = act.tile([P, 2 * H // P, T], AT, tag="H", name=f"H{li}")
            for m, mp, ps in mm_group(li, lambda k, kp: XR[:, k, :], T):
                nc.scalar.activation(
                    Hh[:mp, m, :], ps[:mp, :], AF.Relu, bias=bias_ap(li, m, mp)
                )
            li += 1
            Xn = act.tile([P, H // P, T], AT, tag="X", name=f"X_blk{li}")
            for m, mp, ps in mm_group(li, lambda k, kp: Hh[:, k, :], T):
                nc.vector.scalar_tensor_tensor(
                    Xn[:mp, m, :], ps[:mp, :], bias_ap(li, m, mp), X[:mp, m, :],
                    op0=ALU.add, op1=ALU.add,
                )
            X = Xn
            li += 1

    # ---------------- out_proj -> x_enc (cf: (D, L)) ----------------
    L = D
    vq_lhsT = vq.tile([D + 1, L], F32)
    nc.gpsimd.memset(vq_lhsT[D : D + 1, :], 1.0)
    for m, mp, ps in mm_group(li, lambda k, kp: X[:, k, :], L):
        assert m == 0 and mp == D
        nc.scalar.activation(
            vq_lhsT[:D, :], ps[:D, :], AF.Identity, bias=bias_ap(li, m, mp)
        )
    li += 1

    # x_enc channels-last (L, D) via PE transpose
    tp = psp.tile([D, D], F32, tag="ps", name="tp_xe")
    nc.tensor.transpose(tp[:], vq_lhsT[:D, :], ident[:D, :D])
    xe_cl = vq.tile([L, D], F32)
    nc.scalar.copy(xe_cl[:], tp[:])

    # ---------------- VQ ----------------
    ps_sc = psp.tile([L, CODE_NUM], F32, tag="ps", name="ps_sc")
    nc.tensor.matmul(ps_sc[:], vq_lhsT[:], cbaug_sb[:], start=True, stop=True)
    scores = vq.tile([L, CODE_NUM], F32)
    nc.scalar.copy(scores[:], ps_sc[:])
    max8 = vq.tile([L, 8], F32)
    idx8 = vq.tile([L, 8], U32)
    nc.vector.max(max8[:], scores[:])
    nc.vector.max_index(idx8[:], max8[:], scores[:])
    nc.sync.dma_start(out=idx_d[:], in_=idx8[:, 0:1])

    xq_cl = vq.tile([L, D], F32)
    nc.gpsimd.indirect_dma_start(
        out=xq_cl[:], out_offset=None, in_=cb_d[:],
        in_offset=bass.IndirectOffsetOnAxis(ap=idx8[:, 0:1], axis=0),
    )

    # commit loss partial: per-row sum of (xe - xq)^2 in cl layout
    dd = vq.tile([L, D], F32)
    nc.vector.tensor_sub(dd[:], xe_cl[:], xq_cl[:])
    sqr = vq.tile([L, D], F32)
    sqpart = vq.tile([L, 1], F32)
    nc.scalar.activation(sqr[:], dd[:], AF.Square, accum_out=sqpart[:])
    nc.sync.dma_start(out=sq_d[:], in_=sqpart[:])

    # xq channels-first
    tp2 = psp.tile([D, L], F32, tag="ps", name="tp_xq")
    nc.tensor.transpose(tp2[:], xq_cl[:], ident[:D, :D])
    q_cf = vq.tile([D, L], F32)
    nc.scalar.copy(q_cf[:], tp2[:])

    # ---------------- rotation trick (rows = d, free = l) ----------------
    v = vq_lhsT[:D, :]
    scrA = vq.tile([D, L], F32)
    nv2 = vq.tile([D, 1], F32)
    nc.scalar.activation(scrA[:], v, AF.Square, accum_out=nv2[:])
    nv = vq.tile([D, 1], F32)
    nc.scalar.sqrt(nv[:], nv2[:])
    nc.vector.tensor_scalar_max(nv[:], nv[:], 1e-12)
    inv_nv = vq.tile([D, 1], F32)
    nc.vector.reciprocal(inv_nv[:], nv[:])
    e = vq.tile([D, L], F32)
    nc.vector.tensor_scalar_mul(e[:], v, inv_nv[:, 0:1])

    nq2 = vq.tile([D, 1], F32)
    nc.scalar.activation(scrA[:], q_cf[:], AF.Square, accum_out=nq2[:])
    nq = vq.tile([D, 1], F32)
    nc.scalar.sqrt(nq[:], nq2[:])
    nc.vector.tensor_scalar_max(nq[:], nq[:], 1e-12)
    inv_nq = vq.tile([D, 1], F32)
    nc.vector.reciprocal(inv_nq[:], nq[:])
    qn = vq.tile([D, L], F32)
    nc.vector.tensor_scalar_mul(qn[:], q_cf[:], inv_nq[:, 0:1])

    w_ = vq.tile([D, L], F32)
    nc.vector.tensor_add(w_[:], e[:], qn[:])
    nw2 = vq.tile([D, 1], F32)
    nc.scalar.activation(scrA[:], w_[:], AF.Square, accum_out=nw2[:])
    nw = vq.tile([D, 1], F32)
    nc.scalar.sqrt(nw[:], nw2[:])
    nc.vector.tensor_scalar_max(nw[:], nw[:], 1e-12)
    nwc2 = vq.tile([D, 1], F32)
    nc.vector.tensor_mul(nwc2[:], nw[:], nw[:])
    inv_nw2 = vq.tile([D, 1], F32)
    nc.vector.reciprocal(inv_nw2[:], nwc2[:])

    scrB = vq.tile([D, L], F32)
    nc.vector.tensor_mul(scrB[:], w_[:], v)
    wv = vq.tile([D, 1], F32)
    nc.vector.reduce_sum(wv[:], scrB[:], axis=mybir.AxisListType.X)
    a2 = vq.tile([D, 1], F32)
    nc.vector.tensor_scalar(a2[:], wv[:], inv_nw2[:, 0:1], 2.0, op0=ALU.mult, op1=ALU.mult)
    b2 = vq.tile([D, 1], F32)
    nc.vector.tensor_scalar(b2[:], nv2[:], inv_nv[:, 0:1], 2.0, op0=ALU.mult, op1=ALU.mult)

    t1 = vq.tile([D, L], F32)
    nc.vector.tensor_scalar_mul(t1[:], w_[:], a2[:, 0:1])
    rot = vq.tile([D, L], F32)
    nc.vector.tensor_sub(rot[:], v, t1[:])
    t2 = vq.tile([D, L], F32)
    nc.vector.tensor_scalar_mul(t2[:], qn[:], b2[:, 0:1])
    rot2 = vq.tile([D, L], F32)
    nc.vector.tensor_add(rot2[:], rot[:], t2[:])

    # scaling: per column l, ||xq_cl[l,:]|| / ||xe_cl[l,:]|| (computed in cl layout)
    scrC = vq.tile([L, D], F32)
    ne2 = vq.tile([L, 1], F32)
    nc.scalar.activation(scrC[:], xe_cl[:], AF.Square, accum_out=ne2[:])
    nq2c = vq.tile([L, 1], F32)
    nc.scalar.activation(scrC[:], xq_cl[:], AF.Square, accum_out=nq2c[:])
    inv_ne2 = vq.tile([L, 1], F32)
    nc.vector.reciprocal(inv_ne2[:], ne2[:])
    s2 = vq.tile([L, 1], F32)
    nc.vector.tensor_mul(s2[:], nq2c[:], inv_ne2[:])
    s_cl = vq.tile([L, 1], F32)
    nc.scalar.sqrt(s_cl[:], s2[:])

    tp3 = psp.tile([L, D], F32, tag="ps", name="tp_rot")
    nc.tensor.transpose(tp3[:], rot2[:], ident[:D, :D])
    rot_cl = vq.tile([L, D], F32)
    nc.scalar.copy(rot_cl[:], tp3[:])
    xrot_cl = vq.tile([L, D], F32)
    nc.vector.tensor_scalar_mul(xrot_cl[:], rot_cl[:], s_cl[:, 0:1])

    tp4 = psp.tile([D, L], F32, tag="ps", name="tp_dec")
    nc.tensor.transpose(tp4[:], xrot_cl[:], ident[:L, :L])
    decin_sb = vq.tile([D, L], AT)
    nc.scalar.copy(decin_sb[:], tp4[:])

    # ---------------- decoder ----------------
    T = L
    X = act.tile([P, H // P, T], AT, tag="X", name="X_dec")
    for m, mp, ps in mm_group(li, lambda k, kp: decin_sb[:], T):
        nc.scalar.activation(X[:mp, m, :], ps[:mp, :], AF.Relu, bias=bias_ap(li, m, mp))
    li += 1

    for _lay in range(DOWN):
        for _blk in range(DEPTH):
            XR = act.tile([P, H // P, T], AT, tag="XR", name=f"XR{li}")
            nc.vector.tensor_scalar_max(XR[:], X[:], 0.0)
            Hh = act.tile([P, 2 * H // P, T], AT, tag="H", name=f"H{li}")
            for m, mp, ps in mm_group(li, lambda k, kp: XR[:, k, :], T):
                nc.scalar.activation(
                    Hh[:mp, m, :], ps[:mp, :], AF.Relu, bias=bias_ap(li, m, mp)
                )
            li += 1
            Xn = act.tile([P, H // P, T], AT, tag="X", name=f"X_blk{li}")
            for m, mp, ps in mm_group(li, lambda k, kp: Hh[:, k, :], T):
                nc.vector.scalar_tensor_tensor(
                    Xn[:mp, m, :], ps[:mp, :], bias_ap(li, m, mp), X[:mp, m, :],
                    op0=ALU.add, op1=ALU.add,
                )
            X = Xn
            li += 1

        # up lin (512 -> 1024) then upsample: (1024, T) -> (512, 2T)
        Tout = 2 * T
        Xn = act.tile([P, H // P, Tout], AT, tag="X", name=f"X_up{li}")
        for m, mp, ps in mm_group(li, lambda k, kp: X[:, k, :], T):
            if m < H // P:
                dst = Xn[:mp, m, 0:T]
            else:
                dst = Xn[:mp, m - H // P, T:Tout]
            nc.scalar.activation(dst, ps[:mp, :], AF.Identity, bias=bias_ap(li, m, mp))
        X = Xn
        T = Tout
        li += 1

    # out_proj1 (512 -> 512), relu
    assert T == T0
    Xn = act.tile([P, H // P, T], AT, tag="XR", name="X_op1")
    for m, mp, ps in mm_group(li, lambda k, kp: X[:, k, :], T):
        nc.scalar.activation(Xn[:mp, m, :], ps[:mp, :], AF.Relu, bias=bias_ap(li, m, mp))
    X = Xn
    li += 1

    # out_proj2 (512 -> 63)
    y_sb = act.tile([NFEATS, T], F32, tag="H", name="y_sb")
    for m, mp, ps in mm_group(li, lambda k, kp: X[:, k, :], T):
        assert m == 0 and mp == NFEATS
        nc.scalar.activation(
            y_sb[:mp, :], ps[:mp, :], AF.Identity, bias=bias_ap(li, m, mp)
        )
    li += 1
    assert li == len(_SPECS)
    nc.sync.dma_start(out=y_d[:], in_=y_sb[:])


_PROG_CACHE = {}


def _get_program(repeats=1, hw_loop=1):
    key = (repeats, hw_loop)
    if key not in _PROG_CACHE:
        _PROG_CACHE[key] = _build_program(repeats, hw_loop)
    return _PROG_CACHE[key]


def _np32(x):
    return np.ascontiguousarray(np.asarray(x), dtype=np.float32)


def _ordered_params(enc, dec):
    out = []
    out.append(enc["in_proj"])
    for lay in enc["down"]:
        out.append(lay["lin"])
        for blk in lay["blocks"]:
            out.append(blk["fc1"])
            out.append(blk["fc2"])
    out.append(enc["out_proj"])
    out.append(dec["in_proj"])
    for lay in dec["up"]:
        for blk in lay["blocks"]:
            out.append(blk["fc1"])
            out.append(blk["fc2"])
        out.append(lay["lin"])
    out.append(dec["out_proj1"])
    out.append(dec["out_proj2"])
    return out


def build_in_maps(features, enc_params, dec_params, codebook):
    params = _ordered_params(enc_params, dec_params)
    assert len(params) == len(_SPECS)
    base = {}
    for i, ((w, b), (din, dout, _t, _k)) in enumerate(zip(params, _SPECS)):
        w = _np32(w)
        assert w.shape == (din, dout), (i, w.shape, (din, dout))
        base[f"w{i}"] = w
    bpack = np.zeros((P, _NBCOLS), dtype=np.float32)
    for i, ((w, b), (din, dout, _t, _k)) in enumerate(zip(params, _SPECS)):
        b = _np32(b)
        for m in range(_ceil_div(dout, P)):
            mp = min(P, dout - m * P)
            bpack[:mp, _BCOLS[i] + m] = b[m * P : m * P + mp]
    base["bpack"] = bpack
    cb = _np32(codebook)
    base["cb"] = cb
    cbaug = np.concatenate(
        [2.0 * cb.T, -np.sum(cb.astype(np.float64) ** 2, axis=1, dtype=np.float64)
         .astype(np.float32)[None, :]],
        axis=0,
    ).astype(np.float32)
    base["cbaug"] = np.ascontiguousarray(cbaug)
    feats = _np32(features)
    in_maps = []
    for bidx in range(B):
        m = dict(base)
        m["x0"] = np.ascontiguousarray(feats[bidx].T)
        in_maps.append(m)
    return in_maps


def run_cores(in_maps, repeats=1, hw_loop=1, **kwargs):
    nc = _get_program(repeats, hw_loop)
    return run_bass_kernel_spmd(nc, in_maps, core_ids=list(range(NCORES)), **kwargs)


def kernel(features, enc_params, dec_params, codebook):
    in_maps = build_in_maps(features, enc_params, dec_params, codebook)
    res = run_cores(in_maps)

    ys = np.stack([res.results[i]["y"] for i in range(NCORES)], axis=0)  # (8, 63, 512)
    x_out = np.ascontiguousarray(ys.transpose(0, 2, 1))                  # (8, 512, 63)

    idx_all = np.concatenate(
        [res.results[i]["idx"][:, 0] for i in range(NCORES)]
    ).astype(np.int64)                                                   # (512,)
    sq_all = np.concatenate([res.results[i]["sq"][:, 0] for i in range(NCORES)])

    n_rows = B * D  # B * L, L == D == 64
    commit_loss = np.float32(np.sum(sq_all, dtype=np.float64) / (n_rows * D))

    counts = np.bincount(idx_all, minlength=CODE_NUM).astype(np.float32)
    prob = counts / np.float32(n_rows)
    perplexity = np.float32(
        np.exp(-np.sum(prob * np.log(prob + np.float32(1e-7)), dtype=np.float32))
    )
    return x_out, commit_loss, perplexity


# revision 7
# speedup vs baseline: 2.2581x; 2.2581x over previous
"""Trainium2 Bass kernel for nn_FCVQVae (FC VQ-VAE with rotation trick).

Strategy: data-parallel over batch B=8 across 8 NeuronCores (one batch
element per core). Weights (~90MB fp32) are replicated and streamed from
HBM layer by layer, overlapped with compute via Tile double buffering.
Activations are kept channels-first (channels on partitions, time on the
free axis) so every linear layer is `out = W.T @ X` with W in its natural
(din, dout) layout as the stationary operand.

The VQ nearest-neighbour lookup runs on-device (argmax of 2*x.c - |c|^2 via
DVE max/max_index, codebook gather via indirect DMA); the rotation trick is
done with per-partition vector ops plus two 64x64 PE transposes. Each core
returns its reconstruction (63, 512), VQ indices (64,), and per-row squared
commit error; the host reduces commit loss / perplexity (O(512) scalar work)
and reassembles the full (8, 512, 63) output.
"""

import sys

import numpy as np

sys.path.insert(0, "/opt/trn_rl_repo")

import concourse.bass as bass  # noqa: E402
import concourse.bacc as bacc  # noqa: E402
import concourse.mybir as mybir  # noqa: E402
import concourse.tile as tile  # noqa: E402
from concourse.bass_utils import run_bass_kernel_spmd  # noqa: E402
from concourse.masks import make_identity  # noqa: E402

F32 = mybir.dt.float32
F32R = mybir.dt.float32r
U32 = mybir.dt.uint32
AF = mybir.ActivationFunctionType
ALU = mybir.AluOpType

NFEATS, T0, H, D, CODE_NUM, DOWN, DEPTH, B = 63, 512, 512, 64, 512, 3, 3, 8
NCORES = 8
P = 128

# Matmul dtype mode: "f32" (exact, 4 cyc/row) or "f32r" (1 cyc/row at N>=256).
import os as _os
MM_MODE = _os.environ.get("KERNEL_MM_MODE", "f32")
WP_BUFS = int(_os.environ.get("KERNEL_WP_BUFS", "3"))
ACT_BUFS = int(_os.environ.get("KERNEL_ACT_BUFS", "2"))


def _ceil_div(a, b):
    return (a + b - 1) // b


def _layer_specs():
    """Execution-ordered linear layers: (din, dout, T_out, kind).

    kind: relu | linear | resadd | outproj | decin | uplin | outproj2
    """
    specs = []
    specs.append((NFEATS, H, T0, "relu"))                 # enc in_proj
    t = T0
    for _ in range(DOWN):
        t //= 2
        specs.append((2 * H, H, t, "downlin"))            # enc down lin
        for _ in range(DEPTH):
            specs.append((H, 2 * H, t, "fc1"))            # relu-in, relu evac
            specs.append((2 * H, H, t, "resadd"))         # fc2 + residual
    specs.append((H, D, D, "outproj"))                    # enc out_proj (T=L=64)
    specs.append((D, H, D, "decin"))                      # dec in_proj
    l = D
    for _ in range(DOWN):
        for _ in range(DEPTH):
            specs.append((H, 2 * H, l, "fc1"))
            specs.append((2 * H, H, l, "resadd"))
        specs.append((H, 2 * H, l, "uplin"))              # dec up lin + upsample
        l *= 2
    specs.append((H, H, T0, "relu"))                      # dec out_proj1
    specs.append((H, NFEATS, T0, "outproj2"))             # dec out_proj2
    return specs


_SPECS = _layer_specs()

# bias-pack column offsets per layer (one column per 128-wide m-tile)
_BCOLS = []
_nb = 0
for _din, _dout, _t, _kind in _SPECS:
    _BCOLS.append(_nb)
    _nb += _ceil_div(_dout, P)
_NBCOLS = _nb


def _build_program(repeats=1, hw_loop=1):
    nc = bacc.Bacc(None, target_bir_lowering=False, debug=False)
    AT = F32R if MM_MODE == "f32r" else F32

    x0_d = nc.dram_tensor("x0", [NFEATS, T0], AT, kind="ExternalInput")
    cb_d = nc.dram_tensor("cb", [CODE_NUM, D], F32, kind="ExternalInput")
    cbaug_d = nc.dram_tensor("cbaug", [D + 1, CODE_NUM], F32, kind="ExternalInput")
    bpack_d = nc.dram_tensor("bpack", [P, _NBCOLS], F32, kind="ExternalInput")
    w_drams = [
        nc.dram_tensor(f"w{i}", [s[0], s[1]], AT, kind="ExternalInput")
        for i, s in enumerate(_SPECS)
    ]
    y_d = nc.dram_tensor("y", [NFEATS, T0], F32, kind="ExternalOutput")
    idx_d = nc.dram_tensor("idx", [D, 1], U32, kind="ExternalOutput")
    sq_d = nc.dram_tensor("sq", [D, 1], F32, kind="ExternalOutput")


    with tile.TileContext(nc) as tc:
        with (
            tc.tile_pool(name="const", bufs=1) as const,
            tc.tile_pool(name="wp", bufs=WP_BUFS) as wp,
            tc.tile_pool(name="act", bufs=ACT_BUFS) as act,
            tc.tile_pool(name="vq", bufs=1) as vq,
            tc.tile_pool(name="ps", bufs=8, space="PSUM") as psp,
        ):
            bpack_sb = const.tile([P, _NBCOLS], F32)
            nc.sync.dma_start(out=bpack_sb[:], in_=bpack_d[:])
            cbaug_sb = const.tile([D + 1, CODE_NUM], F32)
            nc.sync.dma_start(out=cbaug_sb[:], in_=cbaug_d[:])
            ident = const.tile([P, P], F32)
            make_identity(nc, ident[:])

            def emit_all():
                for _rep in range(repeats):
                    _emit_forward(
                        nc, tc, wp, act, vq, psp,
                        bpack_sb, cbaug_sb, ident,
                        x0_d, cb_d, w_drams, y_d, idx_d, sq_d, AT,
                    )

            if hw_loop > 1:
                with tc.For_i(0, hw_loop, 1):
                    emit_all()
            else:
                emit_all()

    nc.compile()
    return nc


def _emit_forward(nc, tc, wp, act, vq, psp, bpack_sb, cbaug_sb, ident,
                  x0_d, cb_d, w_drams, y_d, idx_d, sq_d, AT):
    x0_sb = act.tile([NFEATS, T0], AT, tag="x0")
    nc.sync.dma_start(out=x0_sb[:], in_=x0_d[:])

    def load_w(i):
        din, dout, _, _ = _SPECS[i]
        nk = _ceil_div(din, P)
        if din <= P:
            w_sb = wp.tile([din, dout], AT, tag="w", name=f"w_sb{i}")
            nc.sync.dma_start(out=w_sb[:], in_=w_drams[i][:])
            w_sb = w_sb[:].rearrange("p (k m) -> p k m", k=1)
        else:
            assert din % P == 0
            w_sb = wp.tile([P, nk, dout], AT, tag="w", name=f"w_sb{i}")
            nc.sync.dma_start(
                out=w_sb[:], in_=w_drams[i][:].rearrange("(k p) m -> p k m", p=P)
            )
        return w_sb, nk

    def bias_ap(i, m, mp):
        return bpack_sb[:mp, _BCOLS[i] + m : _BCOLS[i] + m + 1]

    def mm_group(i, rhs_fn, T):
        """Yields (m, mp, psum_tile) for layer i with rhs tiles from rhs_fn(k, kp)."""
        din, dout, _, _ = _SPECS[i]
        w_sb, nk = load_w(i)
        for m in range(_ceil_div(dout, P)):
            mp = min(P, dout - m * P)
            ps = psp.tile([P, T], F32, tag="ps", name=f"ps{i}_{m}")
            for k in range(nk):
                kp = min(P, din - k * P)
                lhsT = w_sb[:kp, k, m * P : m * P + mp]
                rhs = rhs_fn(k, kp)
                nc.tensor.matmul(
                    ps[:mp, :], lhsT, rhs,
                    start=(k == 0), stop=(k == nk - 1),
                )
            yield m, mp, ps

    li = 0  # layer index

    # ---------------- encoder ----------------
    # in_proj (63 -> 512), relu
    T = T0
    X = act.tile([P, H // P, T], AT, tag="X", name="X_inp")
    for m, mp, ps in mm_group(li, lambda k, kp: x0_sb[:], T):
        nc.scalar.activation(X[:mp, m, :], ps[:mp, :], AF.Relu, bias=bias_ap(li, m, mp))
    li += 1

    for _lay in range(DOWN):
        Tin, T = T, T // 2
        # down lin (1024 -> 512): k<4 reads even time cols, k>=4 odd
        def rhs_down(k, kp, _X=X, _Tin=Tin):
            half = k // (H // P)
            blk = k % (H // P)
            return _X[:, blk, :].rearrange("p (t two) -> p two t", two=2)[:, half, :]

        Xn = act.tile([P, H // P, T], AT, tag="X", name=f"X_dn{li}")
        for m, mp, ps in mm_group(li, rhs_down, T):
            nc.scalar.activation(
                Xn[:mp, m, :], ps[:mp, :], AF.Identity, bias=bias_ap(li, m, mp)
            )
        X = Xn
        li += 1

        for _blk in range(DEPTH):
            XR = act.tile([P, H // P, T], AT, tag="XR", name=f"XR{li}")
            nc.vector.tensor_scalar_max(XR[:], X[:], 0.0)
            Hh = act.tile([P, 2 * H // P, T], AT, tag="H", name=f"H{li}")
            for m, mp, ps in mm_group(li, lambda k, kp: XR[:, k, :], T):
                nc.scalar.activation(
                    Hh[:mp, m, :], ps[:mp, :], AF.Relu, bias=bias_ap(li, m, mp)
                )
            li += 1
            Xn = act.tile([P, H // P, T], AT, tag="X", name=f"X_blk{li}")
            for m, mp, ps in mm_group(li, lambda k, kp: Hh[:, k, :], T):
                nc.vector.scalar_tensor_tensor(
                    Xn[:mp, m, :], ps[:mp, :], bias_ap(li, m, mp), X[:mp, m, :],
                    op0=ALU.add, op1=ALU.add,
                )
            X = Xn
            li += 1

    # ---------------- out_proj -> x_enc (cf: (D, L)) ----------------
    L = D
    vq_lhsT = vq.tile([D + 1, L], F32)
    nc.gpsimd.memset(vq_lhsT[D : D + 1, :], 1.0)
    for m, mp, ps in mm_group(li, lambda k, kp: X[:, k, :], L):
        assert m == 0 and mp == D
        nc.scalar.activation(
            vq_lhsT[:D, :], ps[:D, :], AF.Identity, bias=bias_ap(li, m, mp)
        )
    li += 1

    # x_enc channels-last (L, D) via PE transpose
    tp = psp.tile([D, D], F32, tag="ps", name="tp_xe")
    nc.tensor.transpose(tp[:], vq_lhsT[:D, :], ident[:D, :D])
    xe_cl = vq.tile([L, D], F32)
    nc.scalar.copy(xe_cl[:], tp[:])

    # ---------------- VQ ----------------
    ps_sc = psp.tile([L, CODE_NUM], F32, tag="ps", name="ps_sc")
    nc.tensor.matmul(ps_sc[:], vq_lhsT[:], cbaug_sb[:], start=True, stop=True)
    scores = vq.tile([L, CODE_NUM], F32)
    nc.scalar.copy(scores[:], ps_sc[:])
    max8 = vq.tile([L, 8], F32)
    idx8 = vq.tile([L, 8], U32)
    nc.vector.max(max8[:], scores[:])
    nc.vector.max_index(idx8[:], max8[:], scores[:])
    nc.sync.dma_start(out=idx_d[:], in_=idx8[:, 0:1])

    xq_cl = vq.tile([L, D], F32)
    nc.gpsimd.indirect_dma_start(
        out=xq_cl[:], out_offset=None, in_=cb_d[:],
        in_offset=bass.IndirectOffsetOnAxis(ap=idx8[:, 0:1], axis=0),
    )

    # commit loss partial: per-row sum of (xe - xq)^2 in cl layout
    dd = vq.tile([L, D], F32)
    nc.vector.tensor_sub(dd[:], xe_cl[:], xq_cl[:])
    sqr = vq.tile([L, D], F32)
    sqpart = vq.tile([L, 1], F32)
    nc.scalar.activation(sqr[:], dd[:], AF.Square, accum_out=sqpart[:])
    nc.sync.dma_start(out=sq_d[:], in_=sqpart[:])

    # xq channels-first
    tp2 = psp.tile([D, L], F32, tag="ps", name="tp_xq")
    nc.tensor.transpose(tp2[:], xq_cl[:], ident[:D, :D])
    q_cf = vq.tile([D, L], F32)
    nc.scalar.copy(q_cf[:], tp2[:])

    # ---------------- rotation trick (rows = d, free = l) ----------------
    v = vq_lhsT[:D, :]
    scrA = vq.tile([D, L], F32)
    nv2 = vq.tile([D, 1], F32)
    nc.scalar.activation(scrA[:], v, AF.Square, accum_out=nv2[:])
    nv = vq.tile([D, 1], F32)
    nc.scalar.sqrt(nv[:], nv2[:])
    nc.vector.tensor_scalar_max(nv[:], nv[:], 1e-12)
    inv_nv = vq.tile([D, 1], F32)
    nc.vector.reciprocal(inv_nv[:], nv[:])
    e = vq.tile([D, L], F32)
    nc.vector.tensor_scalar_mul(e[:], v, inv_nv[:, 0:1])

    nq2 = vq.tile([D, 1], F32)
    nc.scalar.activation(scrA[:], q_cf[:], AF.Square, accum_out=nq2[:])
    nq = vq.tile([D, 1], F32)
    nc.scalar.sqrt(nq[:], nq2[:])
    nc.vector.tensor_scalar_max(nq[:], nq[:], 1e-12)
    inv_nq = vq.tile([D, 1], F32)
    nc.vector.reciprocal(inv_nq[:], nq[:])
    qn = vq.tile([D, L], F32)
    nc.vector.tensor_scalar_mul(qn[:], q_cf[:], inv_nq[:, 0:1])

    w_ = vq.tile([D, L], F32)
    nc.vector.tensor_add(w_[:], e[:], qn[:])
    nw2 = vq.tile([D, 1], F32)
    nc.scalar.activation(scrA[:], w_[:], AF.Square, accum_out=nw2[:])
    nw = vq.tile([D, 1], F32)
    nc.scalar.sqrt(nw[:], nw2[:])
    nc.vector.tensor_scalar_max(nw[:], nw[:], 1e-12)
    nwc2 = vq.tile([D, 1], F32)
    nc.vector.tensor_mul(nwc2[:], nw[:], nw[:])
    inv_nw2 = vq.tile([D, 1], F32)
    nc.vector.reciprocal(inv_nw2[:], nwc2[:])

    scrB = vq.tile([D, L], F32)
    nc.vector.tensor_mul(scrB[:], w_[:], v)
    wv = vq.tile([D, 1], F32)
    nc.vector.reduce_sum(wv[:], scrB[:], axis=mybir.AxisListType.X)
    a2 = vq.tile([D, 1], F32)
    nc.vector.tensor_scalar(a2[:], wv[:], inv_nw2[:, 0:1], 2.0, op0=ALU.mult, op1=ALU.mult)
    b2 = vq.tile([D, 1], F32)
    nc.vector.tensor_scalar(b2[:], nv2[:], inv_nv[:, 0:1], 2.0, op0=ALU.mult, op1=ALU.mult)

    t1 = vq.tile([D, L], F32)
    nc.vector.tensor_scalar_mul(t1[:], w_[:], a2[:, 0:1])
    rot = vq.tile([D, L], F32)
    nc.vector.tensor_sub(rot[:], v, t1[:])
    t2 = vq.tile([D, L], F32)
    nc.vector.tensor_scalar_mul(t2[:], qn[:], b2[:, 0:1])
    rot2 = vq.tile([D, L], F32)
    nc.vector.tensor_add(rot2[:], rot[:], t2[:])

    # scaling: per column l, ||xq_cl[l,:]|| / ||xe_cl[l,:]|| (computed in cl layout)
    scrC = vq.tile([L, D], F32)
    ne2 = vq.tile([L, 1], F32)
    nc.scalar.activation(scrC[:], xe_cl[:], AF.Square, accum_out=ne2[:])
    nq2c = vq.tile([L, 1], F32)
    nc.scalar.activation(scrC[:], xq_cl[:], AF.Square, accum_out=nq2c[:])
    inv_ne2 = vq.tile([L, 1], F32)
    nc.vector.reciprocal(inv_ne2[:], ne2[:])
    s2 = vq.tile([L, 1], F32)
    nc.vector.tensor_mul(s2[:], nq2c[:], inv_ne2[:])
    s_cl = vq.tile([L, 1], F32)
    nc.scalar.sqrt(s_cl[:], s2[:])

    tp3 = psp.tile([L, D], F32, tag="ps", name="tp_rot")
    nc.tensor.transpose(tp3[:], rot2[:], ident[:D, :D])
    rot_cl = vq.tile([L, D], F32)
    nc.scalar.copy(rot_cl[:], tp3[:])
    xrot_cl = vq.tile([L, D], F32)
    nc.vector.tensor_scalar_mul(xrot_cl[:], rot_cl[:], s_cl[:, 0:1])

    tp4 = psp.tile([D, L], F32, tag="ps", name="tp_dec")
    nc.tensor.transpose(tp4[:], xrot_cl[:], ident[:L, :L])
    decin_sb = vq.tile([D, L], AT)
    nc.scalar.copy(decin_sb[:], tp4[:])

    # ---------------- decoder ----------------
    T = L
    X = act.tile([P, H // P, T], AT, tag="X", name="X_dec")
    for m, mp, ps in mm_group(li, lambda k, kp: decin_sb[:], T):
        nc.scalar.activation(X[:mp, m, :], ps[:mp, :], AF.Relu, bias=bias_ap(li, m, mp))
    li += 1

    for _lay in range(DOWN):
        for _blk in range(DEPTH):
            XR = act.tile([P, H // P, T], AT, tag="XR", name=f"XR{li}")
            nc.vector.tensor_scalar_max(XR[:], X[:], 0.0)
            Hh = act.tile([P, 2 * H // P, T], AT, tag="H", name=f"H{li}")
            for m, mp, ps in mm_group(li, lambda k, kp: XR[:, k, :], T):
                nc.scalar.activation(
                    Hh[:mp, m, :], ps[:mp, :], AF.Relu, bias=bias_ap(li, m, mp)
                )
            li += 1
            Xn = act.tile([P, H // P, T], AT, tag="X", name=f"X_blk{li}")
            for m, mp, ps in mm_group(li, lambda k, kp: Hh[:, k, :], T):
                nc.vector.scalar_tensor_tensor(
                    Xn[:mp, m, :], ps[:mp, :], bias_ap(li, m, mp), X[:mp, m, :],
                    op0=ALU.add, op1=ALU.add,
                )
            X = Xn
            li += 1

        # up lin (512 -> 1024) then upsample: (1024, T) -> (512, 2T)
        Tout = 2 * T
        Xn = act.tile([P, H // P, Tout], AT, tag="X", name=f"X_up{li}")
        for m, mp, ps in mm_group(li, lambda k, kp: X[:, k, :], T):
            if m < H // P:
                dst = Xn[:mp, m, 0:T]
            else:
                dst = Xn[:mp, m - H // P, T:Tout]
            nc.scalar.activation(dst, ps[:mp, :], AF.Identity, bias=bias_ap(li, m, mp))
        X = Xn
        T = Tout
        li += 1

    # out_proj1 (512 -> 512), relu
    assert T == T0
    Xn = act.tile([P, H // P, T], AT, tag="XR", name="X_op1")
    for m, mp, ps in mm_group(li, lambda k, kp: X[:, k, :], T):
        nc.scalar.activation(Xn[:mp, m, :], ps[:mp, :], AF.Relu, bias=bias_ap(li, m, mp))
    X = Xn
    li += 1

    # out_proj2 (512 -> 63)
    y_sb = act.tile([NFEATS, T], F32, tag="H", name="y_sb")
    for m, mp, ps in mm_group(li, lambda k, kp: X[:, k, :], T):
        assert m == 0 and mp == NFEATS
        nc.scalar.activation(
            y_sb[:mp, :], ps[:mp, :], AF.Identity, bias=bias_ap(li, m, mp)
        )
    li += 1
    assert li == len(_SPECS)
    nc.sync.dma_start(out=y_d[:], in_=y_sb[:])


_PROG_CACHE = {}


def _get_program(repeats=1, hw_loop=1):
    key = (repeats, hw_loop)
    if key not in _PROG_CACHE:
        _PROG_CACHE[key] = _build_program(repeats, hw_loop)
    return _PROG_CACHE[key]


def _np32(x):
    return np.ascontiguousarray(np.asarray(x), dtype=np.float32)


def _ordered_params(enc, dec):
    out = []
    out.append(enc["in_proj"])
    for lay in enc["down"]:
        out.append(lay["lin"])
        for blk in lay["blocks"]:
            out.append(blk["fc1"])
            out.append(blk["fc2"])
    out.append(enc["out_proj"])
    out.append(dec["in_proj"])
    for lay in dec["up"]:
        for blk in lay["blocks"]:
            out.append(blk["fc1"])
            out.append(blk["fc2"])
        out.append(lay["lin"])
    out.append(dec["out_proj1"])
    out.append(dec["out_proj2"])
    return out


def build_in_maps(features, enc_params, dec_params, codebook):
    params = _ordered_params(enc_params, dec_params)
    assert len(params) == len(_SPECS)
    base = {}
    for i, ((w, b), (din, dout, _t, _k)) in enumerate(zip(params, _SPECS)):
        w = _np32(w)
        assert w.shape == (din, dout), (i, w.shape, (din, dout))
        base[f"w{i}"] = w
    bpack = np.zeros((P, _NBCOLS), dtype=np.float32)
    for i, ((w, b), (din, dout, _t, _k)) in enumerate(zip(params, _SPECS)):
        b = _np32(b)
        for m in range(_ceil_div(dout, P)):
            mp = min(P, dout - m * P)
            bpack[:mp, _BCOLS[i] + m] = b[m * P : m * P + mp]
    base["bpack"] = bpack
    cb = _np32(codebook)
    base["cb"] = cb
    cbaug = np.concatenate(
        [2.0 * cb.T, -np.sum(cb.astype(np.float64) ** 2, axis=1, dtype=np.float64)
         .astype(np.float32)[None, :]],
        axis=0,
    ).astype(np.float32)
    base["cbaug"] = np.ascontiguousarray(cbaug)
    feats = _np32(features)
    in_maps = []
    for bidx in range(B):
        m = dict(base)
        m["x0"] = np.ascontiguousarray(feats[bidx].T)
        in_maps.append(m)
    return in_maps


def run_cores(in_maps, repeats=1, hw_loop=1, **kwargs):
    nc = _get_program(repeats, hw_loop)
    return run_bass_kernel_spmd(nc, in_maps, core_ids=list(range(NCORES)), **kwargs)


def kernel(features, enc_params, dec_params, codebook):
    in_maps = build_in_maps(features, enc_params, dec_params, codebook)
    res = run_cores(in_maps)

    ys = np.stack([res.results[i]["y"] for i in range(NCORES)], axis=0)  # (8, 63, 512)
    x_out = np.ascontiguousarray(ys.transpose(0, 2, 1))                  # (8, 512, 63)

    idx_all = np.concatenate(
        [res.results[i]["idx"][:, 0] for i in range(NCORES)]
    ).astype(np.int64)                                                   # (512,)
    sq_all = np.concatenate([res.results[i]["sq"][:, 0] for i in range(NCORES)])

    n_rows = B * D  # B * L, L == D == 64
    commit_loss = np.float32(np.sum(sq_all, dtype=np.float64) / (n_rows * D))

    counts = np.bincount(idx_all, minlength=CODE_NUM).astype(np.float32)
    prob = counts / np.float32(n_rows)
    perplexity = np.float32(
        np.exp(-np.sum(prob * np.log(prob + np.float32(1e-7)), dtype=np.float32))
    )
    return x_out, commit_loss, perplexity


# revision 8
# speedup vs baseline: 2.3957x; 1.0609x over previous
"""Trainium2 Bass kernel for nn_FCVQVae (FC VQ-VAE with rotation trick).

Strategy: data-parallel over batch B=8 across 8 NeuronCores (one batch
element per core). Weights (~90MB fp32) are replicated and streamed from
HBM layer by layer, overlapped with compute via Tile double buffering.
Activations are kept channels-first (channels on partitions, time on the
free axis) so every linear layer is `out = W.T @ X` with W in its natural
(din, dout) layout as the stationary operand.

The VQ nearest-neighbour lookup runs on-device (argmax of 2*x.c - |c|^2 via
DVE max/max_index, codebook gather via indirect DMA); the rotation trick is
done with per-partition vector ops plus two 64x64 PE transposes. Each core
returns its reconstruction (63, 512), VQ indices (64,), and per-row squared
commit error; the host reduces commit loss / perplexity (O(512) scalar work)
and reassembles the full (8, 512, 63) output.
"""

import sys

import numpy as np

sys.path.insert(0, "/opt/trn_rl_repo")

import concourse.bass as bass  # noqa: E402
import concourse.bacc as bacc  # noqa: E402
import concourse.mybir as mybir  # noqa: E402
import concourse.tile as tile  # noqa: E402
from concourse.bass_utils import run_bass_kernel_spmd  # noqa: E402
from concourse.masks import make_identity  # noqa: E402

F32 = mybir.dt.float32
F32R = mybir.dt.float32r
U32 = mybir.dt.uint32
AF = mybir.ActivationFunctionType
ALU = mybir.AluOpType

NFEATS, T0, H, D, CODE_NUM, DOWN, DEPTH, B = 63, 512, 512, 64, 512, 3, 3, 8
NCORES = 8
P = 128

# Matmul dtype mode: "f32" (exact, 4 cyc/row) or "f32r" (1 cyc/row at N>=256).
import os as _os
MM_MODE = _os.environ.get("KERNEL_MM_MODE", "f32")
WP_BUFS = int(_os.environ.get("KERNEL_WP_BUFS", "3"))
ACT_BUFS = int(_os.environ.get("KERNEL_ACT_BUFS", "2"))
GROUP_COLS = int(_os.environ.get("KERNEL_GROUP_COLS", "6144"))


def _ceil_div(a, b):
    return (a + b - 1) // b


def _layer_specs():
    """Execution-ordered linear layers: (din, dout, T_out, kind).

    kind: relu | linear | resadd | outproj | decin | uplin | outproj2
    """
    specs = []
    specs.append((NFEATS, H, T0, "relu"))                 # enc in_proj
    t = T0
    for _ in range(DOWN):
        t //= 2
        specs.append((2 * H, H, t, "downlin"))            # enc down lin
        for _ in range(DEPTH):
            specs.append((H, 2 * H, t, "fc1"))            # relu-in, relu evac
            specs.append((2 * H, H, t, "resadd"))         # fc2 + residual
    specs.append((H, D, D, "outproj"))                    # enc out_proj (T=L=64)
    specs.append((D, H, D, "decin"))                      # dec in_proj
    l = D
    for _ in range(DOWN):
        for _ in range(DEPTH):
            specs.append((H, 2 * H, l, "fc1"))
            specs.append((2 * H, H, l, "resadd"))
        specs.append((H, 2 * H, l, "uplin"))              # dec up lin + upsample
        l *= 2
    specs.append((H, H, T0, "relu"))                      # dec out_proj1
    specs.append((H, NFEATS, T0, "outproj2"))             # dec out_proj2
    return specs


_SPECS = _layer_specs()

# bias-pack column offsets per layer (one column per 128-wide m-tile)
_BCOLS = []
_nb = 0
for _din, _dout, _t, _kind in _SPECS:
    _BCOLS.append(_nb)
    _nb += _ceil_div(_dout, P)
_NBCOLS = _nb

# wpack layout: each layer i occupies _WOFF[i] .. _WOFF[i]+_WCOLS[i] columns of a
# (128, _NWCOLS) fp32 array; element [p, _WOFF[i] + k*dout + m] = W_i[k*128+p, m]
# (zero-padded rows when din is not a multiple of 128).
_WCOLS = [_ceil_div(s[0], P) * s[1] for s in _SPECS]
_WOFF = []
_off = 0
for _c in _WCOLS:
    _WOFF.append(_off)
    _off += _c
_NWCOLS = _off


def _wgroups():
    """Greedy grouping of consecutive layers into single weight DMAs."""
    groups = []
    cur = []
    cur_cols = 0
    for i, c in enumerate(_WCOLS):
        if cur and cur_cols + c > GROUP_COLS:
            groups.append(cur)
            cur = []
            cur_cols = 0
        cur.append(i)
        cur_cols += c
    if cur:
        groups.append(cur)
    return groups


def _build_program(repeats=1, hw_loop=1):
    nc = bacc.Bacc(None, target_bir_lowering=False, debug=False)
    AT = F32R if MM_MODE == "f32r" else F32

    x0_d = nc.dram_tensor("x0", [NFEATS, T0], AT, kind="ExternalInput")
    cb_d = nc.dram_tensor("cb", [CODE_NUM, D], F32, kind="ExternalInput")
    cbaug_d = nc.dram_tensor("cbaug", [D + 1, CODE_NUM], F32, kind="ExternalInput")
    bpack_d = nc.dram_tensor("bpack", [P, _NBCOLS], F32, kind="ExternalInput")
    wpack_d = nc.dram_tensor("wpack", [P, _NWCOLS], AT, kind="ExternalInput")
    y_d = nc.dram_tensor("y", [NFEATS, T0], F32, kind="ExternalOutput")
    idx_d = nc.dram_tensor("idx", [D, 1], U32, kind="ExternalOutput")
    sq_d = nc.dram_tensor("sq", [D, 1], F32, kind="ExternalOutput")


    with tile.TileContext(nc) as tc:
        with (
            tc.tile_pool(name="const", bufs=1) as const,
            tc.tile_pool(name="wp", bufs=WP_BUFS) as wp,
            tc.tile_pool(name="act", bufs=ACT_BUFS) as act,
            tc.tile_pool(name="vq", bufs=1) as vq,
            tc.tile_pool(name="ps", bufs=8, space="PSUM") as psp,
        ):
            bpack_sb = const.tile([P, _NBCOLS], F32)
            nc.sync.dma_start(out=bpack_sb[:], in_=bpack_d[:])
            cbaug_sb = const.tile([D + 1, CODE_NUM], F32)
            nc.sync.dma_start(out=cbaug_sb[:], in_=cbaug_d[:])
            ident = const.tile([P, P], F32)
            make_identity(nc, ident[:])

            def emit_all():
                for _rep in range(repeats):
                    _emit_forward(
                        nc, tc, wp, act, vq, psp,
                        bpack_sb, cbaug_sb, ident,
                        x0_d, cb_d, wpack_d, y_d, idx_d, sq_d, AT,
                    )

            if hw_loop > 1:
                with tc.For_i(0, hw_loop, 1):
                    emit_all()
            else:
                emit_all()

    nc.compile()
    return nc


def _emit_forward(nc, tc, wp, act, vq, psp, bpack_sb, cbaug_sb, ident,
                  x0_d, cb_d, wpack_d, y_d, idx_d, sq_d, AT):
    x0_sb = act.tile([NFEATS, T0], AT, tag="x0")
    nc.sync.dma_start(out=x0_sb[:], in_=x0_d[:])

    groups = _wgroups()
    layer_group = {}
    for gi, g in enumerate(groups):
        for i in g:
            layer_group[i] = gi
    group_tiles = {}

    def load_w(i):
        din, dout, _, _ = _SPECS[i]
        nk = _ceil_div(din, P)
        gi = layer_group[i]
        if gi not in group_tiles:
            g = groups[gi]
            c0 = _WOFF[g[0]]
            cols = _WOFF[g[-1]] + _WCOLS[g[-1]] - c0
            gt = wp.tile([P, cols], AT, tag="w", name=f"wg{gi}")
            nc.sync.dma_start(out=gt[:], in_=wpack_d[:, c0 : c0 + cols])
            group_tiles[gi] = (gt, c0)
        gt, c0 = group_tiles[gi]
        w_sb = gt[:, _WOFF[i] - c0 : _WOFF[i] - c0 + nk * dout].rearrange(
            "p (k m) -> p k m", k=nk
        )
        return w_sb, nk

    def bias_ap(i, m, mp):
        return bpack_sb[:mp, _BCOLS[i] + m : _BCOLS[i] + m + 1]

    def mm_group(i, rhs_fn, T):
        """Yields (m, mp, psum_tile) for layer i with rhs tiles from rhs_fn(k, kp)."""
        din, dout, _, _ = _SPECS[i]
        w_sb, nk = load_w(i)
        for m in range(_ceil_div(dout, P)):
            mp = min(P, dout - m * P)
            ps = psp.tile([P, T], F32, tag="ps", name=f"ps{i}_{m}")
            for k in range(nk):
                kp = min(P, din - k * P)
                lhsT = w_sb[:kp, k, m * P : m * P + mp]
                rhs = rhs_fn(k, kp)
                nc.tensor.matmul(
                    ps[:mp, :], lhsT, rhs,
                    start=(k == 0), stop=(k == nk - 1),
                )
            yield m, mp, ps

    li = 0  # layer index

    # ---------------- encoder ----------------
    # in_proj (63 -> 512), relu
    T = T0
    X = act.tile([P, H // P, T], AT, tag="X", name="X_inp")
    for m, mp, ps in mm_group(li, lambda k, kp: x0_sb[:], T):
        nc.scalar.activation(X[:mp, m, :], ps[:mp, :], AF.Relu, bias=bias_ap(li, m, mp))
    li += 1

    for _lay in range(DOWN):
        Tin, T = T, T // 2
        # down lin (1024 -> 512): k<4 reads even time cols, k>=4 odd
        def rhs_down(k, kp, _X=X, _Tin=Tin):
            half = k // (H // P)
            blk = k % (H // P)
            return _X[:, blk, :].rearrange("p (t two) -> p two t", two=2)[:, half, :]

        Xn = act.tile([P, H // P, T], AT, tag="X", name=f"X_dn{li}")
        for m, mp, ps in mm_group(li, rhs_down, T):
            nc.scalar.activation(
                Xn[:mp, m, :], ps[:mp, :], AF.Identity, bias=bias_ap(li, m, mp)
            )
        X = Xn
        li += 1

        for _blk in range(DEPTH):
            XR = act.tile([P, H // P, T], AT, tag="XR", name=f"XR{li}")
            nc.vector.tensor_scalar_max(XR[:], X[:], 0.0)
            Hh = act.tile([P, 2 * H // P, T], AT, tag="H", name=f"H{li}")
            for m, mp, ps in mm_group(li, lambda k, kp: XR[:, k, :], T):
                nc.scalar.activation(
                    Hh[:mp, m, :], ps[:mp, :], AF.Relu, bias=bias_ap(li, m, mp)
                )
            li += 1
            Xn = act.tile([P, H // P, T], AT, tag="X", name=f"X_blk{li}")
            for m, mp, ps in mm_group(li, lambda k, kp: Hh[:, k, :], T):
                nc.vector.scalar_tensor_tensor(
                    Xn[:mp, m, :], ps[:mp, :], bias_ap(li, m, mp), X[:mp, m, :],
                    op0=ALU.add, op1=ALU.add,
                )
            X = Xn
            li += 1

    # ---------------- out_proj -> x_enc (cf: (D, L)) ----------------
    L = D
    vq_lhsT = vq.tile([D + 1, L], F32)
    nc.gpsimd.memset(vq_lhsT[D : D + 1, :], 1.0)
    for m, mp, ps in mm_group(li, lambda k, kp: X[:, k, :], L):
        assert m == 0 and mp == D
        nc.scalar.activation(
            vq_lhsT[:D, :], ps[:D, :], AF.Identity, bias=bias_ap(li, m, mp)
        )
    li += 1

    # x_enc channels-last (L, D) via PE transpose
    tp = psp.tile([D, D], F32, tag="ps", name="tp_xe")
    nc.tensor.transpose(tp[:], vq_lhsT[:D, :], ident[:D, :D])
    xe_cl = vq.tile([L, D], F32)
    nc.scalar.copy(xe_cl[:], tp[:])

    # ---------------- VQ ----------------
    ps_sc = psp.tile([L, CODE_NUM], F32, tag="ps", name="ps_sc")
    nc.tensor.matmul(ps_sc[:], vq_lhsT[:], cbaug_sb[:], start=True, stop=True)
    scores = vq.tile([L, CODE_NUM], F32)
    nc.scalar.copy(scores[:], ps_sc[:])
    max8 = vq.tile([L, 8], F32)
    idx8 = vq.tile([L, 8], U32)
    nc.vector.max(max8[:], scores[:])
    nc.vector.max_index(idx8[:], max8[:], scores[:])
    nc.sync.dma_start(out=idx_d[:], in_=idx8[:, 0:1])

    xq_cl = vq.tile([L, D], F32)
    nc.gpsimd.indirect_dma_start(
        out=xq_cl[:], out_offset=None, in_=cb_d[:],
        in_offset=bass.IndirectOffsetOnAxis(ap=idx8[:, 0:1], axis=0),
    )

    # commit loss partial: per-row sum of (xe - xq)^2 in cl layout
    dd = vq.tile([L, D], F32)
    nc.vector.tensor_sub(dd[:], xe_cl[:], xq_cl[:])
    sqr = vq.tile([L, D], F32)
    sqpart = vq.tile([L, 1], F32)
    nc.scalar.activation(sqr[:], dd[:], AF.Square, accum_out=sqpart[:])
    nc.sync.dma_start(out=sq_d[:], in_=sqpart[:])

    # xq channels-first
    tp2 = psp.tile([D, L], F32, tag="ps", name="tp_xq")
    nc.tensor.transpose(tp2[:], xq_cl[:], ident[:D, :D])
    q_cf = vq.tile([D, L], F32)
    nc.scalar.copy(q_cf[:], tp2[:])

    # ---------------- rotation trick (rows = d, free = l) ----------------
    v = vq_lhsT[:D, :]
    scrA = vq.tile([D, L], F32)
    nv2 = vq.tile([D, 1], F32)
    nc.scalar.activation(scrA[:], v, AF.Square, accum_out=nv2[:])
    nv = vq.tile([D, 1], F32)
    nc.scalar.sqrt(nv[:], nv2[:])
    nc.vector.tensor_scalar_max(nv[:], nv[:], 1e-12)
    inv_nv = vq.tile([D, 1], F32)
    nc.vector.reciprocal(inv_nv[:], nv[:])
    e = vq.tile([D, L], F32)
    nc.vector.tensor_scalar_mul(e[:], v, inv_nv[:, 0:1])

    nq2 = vq.tile([D, 1], F32)
    nc.scalar.activation(scrA[:], q_cf[:], AF.Square, accum_out=nq2[:])
    nq = vq.tile([D, 1], F32)
    nc.scalar.sqrt(nq[:], nq2[:])
    nc.vector.tensor_scalar_max(nq[:], nq[:], 1e-12)
    inv_nq = vq.tile([D, 1], F32)
    nc.vector.reciprocal(inv_nq[:], nq[:])
    qn = vq.tile([D, L], F32)
    nc.vector.tensor_scalar_mul(qn[:], q_cf[:], inv_nq[:, 0:1])

    w_ = vq.tile([D, L], F32)
    nc.vector.tensor_add(w_[:], e[:], qn[:])
    nw2 = vq.tile([D, 1], F32)
    nc.scalar.activation(scrA[:], w_[:], AF.Square, accum_out=nw2[:])
    nw = vq.tile([D, 1], F32)
    nc.scalar.sqrt(nw[:], nw2[:])
    nc.vector.tensor_scalar_max(nw[:], nw[:], 1e-12)
    nwc2 = vq.tile([D, 1], F32)
    nc.vector.tensor_mul(nwc2[:], nw[:], nw[:])
    inv_nw2 = vq.tile([D, 1], F32)
    nc.vector.reciprocal(inv_nw2[:], nwc2[:])

    scrB = vq.tile([D, L], F32)
    nc.vector.tensor_mul(scrB[:], w_[:], v)
    wv = vq.tile([D, 1], F32)
    nc.vector.reduce_sum(wv[:], scrB[:], axis=mybir.AxisListType.X)
    a2 = vq.tile([D, 1], F32)
    nc.vector.tensor_scalar(a2[:], wv[:], inv_nw2[:, 0:1], 2.0, op0=ALU.mult, op1=ALU.mult)
    b2 = vq.tile([D, 1], F32)
    nc.vector.tensor_scalar(b2[:], nv2[:], inv_nv[:, 0:1], 2.0, op0=ALU.mult, op1=ALU.mult)

    t1 = vq.tile([D, L], F32)
    nc.vector.tensor_scalar_mul(t1[:], w_[:], a2[:, 0:1])
    rot = vq.tile([D, L], F32)
    nc.vector.tensor_sub(rot[:], v, t1[:])
    t2 = vq.tile([D, L], F32)
    nc.vector.tensor_scalar_mul(t2[:], qn[:], b2[:, 0:1])
    rot2 = vq.tile([D, L], F32)
    nc.vector.tensor_add(rot2[:], rot[:], t2[:])

    # scaling: per column l, ||xq_cl[l,:]|| / ||xe_cl[l,:]|| (computed in cl layout)
    scrC = vq.tile([L, D], F32)
    ne2 = vq.tile([L, 1], F32)
    nc.scalar.activation(scrC[:], xe_cl[:], AF.Square, accum_out=ne2[:])
    nq2c = vq.tile([L, 1], F32)
    nc.scalar.activation(scrC[:], xq_cl[:], AF.Square, accum_out=nq2c[:])
    inv_ne2 = vq.tile([L, 1], F32)
    nc.vector.reciprocal(inv_ne2[:], ne2[:])
    s2 = vq.tile([L, 1], F32)
    nc.vector.tensor_mul(s2[:], nq2c[:], inv_ne2[:])
    s_cl = vq.tile([L, 1], F32)
    nc.scalar.sqrt(s_cl[:], s2[:])

    tp3 = psp.tile([L, D], F32, tag="ps", name="tp_rot")
    nc.tensor.transpose(tp3[:], rot2[:], ident[:D, :D])
    rot_cl = vq.tile([L, D], F32)
    nc.scalar.copy(rot_cl[:], tp3[:])
    xrot_cl = vq.tile([L, D], F32)
    nc.vector.tensor_scalar_mul(xrot_cl[:], rot_cl[:], s_cl[:, 0:1])

    tp4 = psp.tile([D, L], F32, tag="ps", name="tp_dec")
    nc.tensor.transpose(tp4[:], xrot_cl[:], ident[:L, :L])
    decin_sb = vq.tile([D, L], AT)
    nc.scalar.copy(decin_sb[:], tp4[:])

    # ---------------- decoder ----------------
    T = L
    X = act.tile([P, H // P, T], AT, tag="X", name="X_dec")
    for m, mp, ps in mm_group(li, lambda k, kp: decin_sb[:], T):
        nc.scalar.activation(X[:mp, m, :], ps[:mp, :], AF.Relu, bias=bias_ap(li, m, mp))
    li += 1

    for _lay in range(DOWN):
        for _blk in range(DEPTH):
            XR = act.tile([P, H // P, T], AT, tag="XR", name=f"XR{li}")
            nc.vector.tensor_scalar_max(XR[:], X[:], 0.0)
            Hh = act.tile([P, 2 * H // P, T], AT, tag="H", name=f"H{li}")
            for m, mp, ps in mm_group(li, lambda k, kp: XR[:, k, :], T):
                nc.scalar.activation(
                    Hh[:mp, m, :], ps[:mp, :], AF.Relu, bias=bias_ap(li, m, mp)
                )
            li += 1
            Xn = act.tile([P, H // P, T], AT, tag="X", name=f"X_blk{li}")
            for m, mp, ps in mm_group(li, lambda k, kp: Hh[:, k, :], T):
                nc.vector.scalar_tensor_tensor(
                    Xn[:mp, m, :], ps[:mp, :], bias_ap(li, m, mp), X[:mp, m, :],
                    op0=ALU.add, op1=ALU.add,
                )
            X = Xn
            li += 1

        # up lin (512 -> 1024) then upsample: (1024, T) -> (512, 2T)
        Tout = 2 * T
        Xn = act.tile([P, H // P, Tout], AT, tag="X", name=f"X_up{li}")
        for m, mp, ps in mm_group(li, lambda k, kp: X[:, k, :], T):
            if m < H // P:
                dst = Xn[:mp, m, 0:T]
            else:
                dst = Xn[:mp, m - H // P, T:Tout]
            nc.scalar.activation(dst, ps[:mp, :], AF.Identity, bias=bias_ap(li, m, mp))
        X = Xn
        T = Tout
        li += 1

    # out_proj1 (512 -> 512), relu
    assert T == T0
    Xn = act.tile([P, H // P, T], AT, tag="XR", name="X_op1")
    for m, mp, ps in mm_group(li, lambda k, kp: X[:, k, :], T):
        nc.scalar.activation(Xn[:mp, m, :], ps[:mp, :], AF.Relu, bias=bias_ap(li, m, mp))
    X = Xn
    li += 1

    # out_proj2 (512 -> 63)
    y_sb = act.tile([NFEATS, T], F32, tag="H", name="y_sb")
    for m, mp, ps in mm_group(li, lambda k, kp: X[:, k, :], T):
        assert m == 0 and mp == NFEATS
        nc.scalar.activation(
            y_sb[:mp, :], ps[:mp, :], AF.Identity, bias=bias_ap(li, m, mp)
        )
    li += 1
    assert li == len(_SPECS)
    nc.sync.dma_start(out=y_d[:], in_=y_sb[:])


_PROG_CACHE = {}


def _get_program(repeats=1, hw_loop=1):
    key = (repeats, hw_loop)
    if key not in _PROG_CACHE:
        _PROG_CACHE[key] = _build_program(repeats, hw_loop)
    return _PROG_CACHE[key]


def _np32(x):
    return np.ascontiguousarray(np.asarray(x), dtype=np.float32)


def _ordered_params(enc, dec):
    out = []
    out.append(enc["in_proj"])
    for lay in enc["down"]:
        out.append(lay["lin"])
        for blk in lay["blocks"]:
            out.append(blk["fc1"])
            out.append(blk["fc2"])
    out.append(enc["out_proj"])
    out.append(dec["in_proj"])
    for lay in dec["up"]:
        for blk in lay["blocks"]:
            out.append(blk["fc1"])
            out.append(blk["fc2"])
        out.append(lay["lin"])
    out.append(dec["out_proj1"])
    out.append(dec["out_proj2"])
    return out


def build_in_maps(features, enc_params, dec_params, codebook):
    params = _ordered_params(enc_params, dec_params)
    assert len(params) == len(_SPECS)
    base = {}
    wpack = np.zeros((P, _NWCOLS), dtype=np.float32)
    for i, ((w, b), (din, dout, _t, _k)) in enumerate(zip(params, _SPECS)):
        w = _np32(w)
        assert w.shape == (din, dout), (i, w.shape, (din, dout))
        nk = _ceil_div(din, P)
        wpad = np.zeros((nk * P, dout), dtype=np.float32)
        wpad[:din] = w
        # [p, k*dout + m] = W[k*128+p, m]
        wpack[:, _WOFF[i] : _WOFF[i] + nk * dout] = (
            wpad.reshape(nk, P, dout).transpose(1, 0, 2).reshape(P, nk * dout)
        )
    base["wpack"] = wpack
    bpack = np.zeros((P, _NBCOLS), dtype=np.float32)
    for i, ((w, b), (din, dout, _t, _k)) in enumerate(zip(params, _SPECS)):
        b = _np32(b)
        for m in range(_ceil_div(dout, P)):
            mp = min(P, dout - m * P)
            bpack[:mp, _BCOLS[i] + m] = b[m * P : m * P + mp]
    base["bpack"] = bpack
    cb = _np32(codebook)
    base["cb"] = cb
    cbaug = np.concatenate(
        [2.0 * cb.T, -np.sum(cb.astype(np.float64) ** 2, axis=1, dtype=np.float64)
         .astype(np.float32)[None, :]],
        axis=0,
    ).astype(np.float32)
    base["cbaug"] = np.ascontiguousarray(cbaug)
    feats = _np32(features)
    in_maps = []
    for bidx in range(B):
        m = dict(base)
        m["x0"] = np.ascontiguousarray(feats[bidx].T)
        in_maps.append(m)
    return in_maps


def run_cores(in_maps, repeats=1, hw_loop=1, **kwargs):
    nc = _get_program(repeats, hw_loop)
    return run_bass_kernel_spmd(nc, in_maps, core_ids=list(range(NCORES)), **kwargs)


def kernel(features, enc_params, dec_params, codebook):
    in_maps = build_in_maps(features, enc_params, dec_params, codebook)
    res = run_cores(in_maps)

    ys = np.stack([res.results[i]["y"] for i in range(NCORES)], axis=0)  # (8, 63, 512)
    x_out = np.ascontiguousarray(ys.transpose(0, 2, 1))                  # (8, 512, 63)

    idx_all = np.concatenate(
        [res.results[i]["idx"][:, 0] for i in range(NCORES)]
    ).astype(np.int64)                                                   # (512,)
    sq_all = np.concatenate([res.results[i]["sq"][:, 0] for i in range(NCORES)])

    n_rows = B * D  # B * L, L == D == 64
    commit_loss = np.float32(np.sum(sq_all, dtype=np.float64) / (n_rows * D))

    counts = np.bincount(idx_all, minlength=CODE_NUM).astype(np.float32)
    prob = counts / np.float32(n_rows)
    perplexity = np.float32(
        np.exp(-np.sum(prob * np.log(prob + np.float32(1e-7)), dtype=np.float32))
    )
    return x_out, commit_loss, perplexity
